# revision 10
# baseline (speedup 1.0000x reference)
"""Trainium2 Bass kernel for nn_CreateOverlappingWindows.

out[b, t, w*C + c] = x_padded[b, t + w, c]  (SAME zero padding, n_context=9)

Flattening (w, c) -> 494 contiguous values, each output row is a contiguous
494-element window of the zero-padded flattened input:
    out[b, t, :] = xpad_flat[b, t*C : t*C + W*C]

Strategy (memory-regime): bf16 end-to-end (tolerance 2e-2; bf16 keeps f32's
exponent range so rel err <= 2^-9 ~ 2e-3), and run TWO independent
output-write paths concurrently, each bound by a different resource:

  * SBUF path (batches 0,1): SWDGE loads the padded input into SBUF
    (50 partitions x 1508-elem overlapping slices), DVE expands the windows
    into a dense tile (int32-viewed copies, ~5.2us/batch), SWDGE streams it
    out with 39.5 KB fully-contiguous descriptors. SWDGE paces ~75ns/desc,
    so 50 descriptors/batch ~= 3.8us/batch.
  * Direct path (batches 2,3): one HWDGE ring each (sync / scalar) does
    the window-gather DRAM->DRAM with 988 B descriptors straight from the
    padded input (HWDGE descriptor-rate bound, ~10.4ns/desc -> ~21us).

Sharding: pure data parallel - batch 32 split 4-per-core across 8 cores.
"""

import sys

sys.path.insert(0, "/opt/trn_rl_repo")

import ml_dtypes
import numpy as np
from concourse import bass, mybir
from concourse.ap import AP
from concourse.bass_utils import run_bass_kernel_spmd

_BF16 = mybir.dt.bfloat16
_I32 = mybir.dt.int32
_NPBF16 = ml_dtypes.bfloat16

_NCORES = 8
_B, _T, _C = 32, 2000, 26
_NCTX = 9
_W = 2 * _NCTX + 1  # 19
_WC = _W * _C  # 494
_PAD = _NCTX * _C  # 234
_BPC = _B // _NCORES  # 4 batches per core
_NP = _T * _C + 2 * _PAD  # 52468 padded flat length per batch
_TWC = _T * _WC  # 988000

_NSB = 2  # batches on the SBUF/SWDGE path; the rest go direct HWDGE

_PPB = 50  # partitions per batch
_RPP = 40  # output rows per partition  (50 * 40 = 2000)
_STEP = _RPP * _C  # 1040: flat-input stride between partition slices
_SEG = _STEP + (_WC - _C)  # 1508: slice length incl. 468-element halo
_FI = _NSB * _SEG  # free elems/partition, input tile
_RW = _RPP * _WC  # 19760: dense output elems/partition/batch
_FO = _NSB * _RW  # free elems/partition, output tile

_nc_cache = None


def _build():
    global _nc_cache
    if _nc_cache is not None:
        return _nc_cache
    nc = bass.Bass()
    xp = nc.declare_dram_parameter("xp", [_BPC, _NP], _BF16, isOutput=False)
    out = nc.declare_dram_parameter("out", [_BPC, _T, _WC], _BF16, isOutput=True)

    with (
        nc.sbuf_tensor([128, _FI], _BF16) as tin,
        nc.sbuf_tensor([128, _FO], _BF16) as tout,
        nc.Block() as block,
        nc.semaphore("l0") as l0,
        nc.semaphore("l1") as l1,
        nc.semaphore("es") as es,
        nc.semaphore("ss") as ss,
        nc.semaphore("hs") as hs,
    ):
        lsem = [l0, l1]

        @block.gpsimd
        def _(e):
            for b in range(_NSB):
                e.dma_start(
                    out=AP(tin, b * _SEG, [[_FI, _PPB], [1, _SEG]]),
                    in_=AP(xp, b * _NP, [[_STEP, _PPB], [1, _SEG]]),
                ).then_inc(lsem[b], 16)
            for b in range(_NSB):
                e.wait_ge(es, b + 1)
                e.dma_start(
                    out=AP(out, b * _TWC, [[_RW, _PPB], [1, _RW]]),
                    in_=AP(tout, b * _RW, [[_FO, _PPB], [1, _RW]]),
                ).then_inc(ss, 16)
            e.wait_ge(ss, 16 * _NSB)

        @block.vector
        def _(v):
            for b in range(_NSB):
                v.wait_ge(lsem[b], 16)
                v.tensor_copy(
                    out=AP(
                        tout, b * _RW, [[_FO, _PPB], [_WC, _RPP], [1, _WC]]
                    ).bitcast(_I32),
                    in_=AP(
                        tin, b * _SEG, [[_FI, _PPB], [_C, _RPP], [1, _WC]]
                    ).bitcast(_I32),
                ).then_inc(es, 1)

        @block.sync
        def _(e):
            b = _NSB  # batch 2: direct DRAM->DRAM window gather
            e.dma_start(
                out=AP(out, b * _TWC, [[_WC, _T], [1, _WC]]),
                in_=AP(xp, b * _NP, [[_C, _T], [1, _WC]]),
            ).then_inc(hs, 16)
            e.wait_ge(hs, 32)

        @block.scalar
        def _(e):
            b = _NSB + 1  # batch 3: direct DRAM->DRAM window gather
            e.dma_start(
                out=AP(out, b * _TWC, [[_WC, _T], [1, _WC]]),
                in_=AP(xp, b * _NP, [[_C, _T], [1, _WC]]),
            ).then_inc(hs, 16)
            e.wait_ge(hs, 32)

    _nc_cache = nc
    return nc


def _make_in_maps(x: np.ndarray) -> list[dict]:
    """x: [B, T, C] float32 -> per-core padded bf16 flat inputs."""
    xb = np.asarray(x, dtype=np.float32).astype(_NPBF16)
    xpad = np.zeros((_B, _NP), _NPBF16)
    xpad[:, _PAD : _PAD + _T * _C] = xb.reshape(_B, _T * _C)
    return [
        {"xp": np.ascontiguousarray(xpad[i * _BPC : (i + 1) * _BPC])}
        for i in range(_NCORES)
    ]


def _gather_out(results) -> np.ndarray:
    return np.concatenate(
        [np.asarray(r["out"]).astype(np.float32) for r in results], axis=0
    ).reshape(_B, _T, _WC)


def kernel(x: np.ndarray) -> np.ndarray:
    assert np.asarray(x).shape == (_B, _T, _C)
    nc = _build()
    res = run_bass_kernel_spmd(nc, _make_in_maps(x), list(range(_NCORES)))
    return _gather_out(res.results)


# revision 11
# speedup vs baseline: 1.1108x; 1.1108x over previous
"""Trainium2 Bass kernel for nn_CreateOverlappingWindows.

out[b, t, w*C + c] = x_padded[b, t + w, c]  (SAME zero padding, n_context=9)

Flattening (w, c) -> 494 contiguous values, each output row is a contiguous
494-element window of the zero-padded flattened input:
    out[b, t, :] = xpad_flat[b, t*C : t*C + W*C]

Strategy (memory-regime): bf16 end-to-end (tolerance 2e-2; bf16 keeps f32's
exponent range so rel err <= 2^-9 ~ 2e-3), and run TWO independent
output-write paths concurrently, each bound by a different resource:

  * SBUF path (batches 0,1): SWDGE loads the padded input into SBUF
    (100 partitions x 988-elem overlapping slices), DVE expands the windows
    into a dense tile (int32-viewed copies, ~2.7us/batch), SWDGE streams it
    out with 19.8 KB fully-contiguous descriptors (the per-descriptor
    throughput sweet spot; >~20KB descriptors run at half rate).
  * Direct path (batches 2,3): one HWDGE ring each (sync / scalar) does
    the window-gather DRAM->DRAM with 988 B descriptors straight from the
    padded input (HWDGE descriptor-rate bound, ~10.4ns/desc -> ~21us).

Sharding: pure data parallel - batch 32 split 4-per-core across 8 cores.
"""

import sys

sys.path.insert(0, "/opt/trn_rl_repo")

import ml_dtypes
import numpy as np
from concourse import bass, mybir
from concourse.ap import AP
from concourse.bass_utils import run_bass_kernel_spmd

_BF16 = mybir.dt.bfloat16
_I32 = mybir.dt.int32
_NPBF16 = ml_dtypes.bfloat16

_NCORES = 8
_B, _T, _C = 32, 2000, 26
_NCTX = 9
_W = 2 * _NCTX + 1  # 19
_WC = _W * _C  # 494
_PAD = _NCTX * _C  # 234
_BPC = _B // _NCORES  # 4 batches per core
_NP = _T * _C + 2 * _PAD  # 52468 padded flat length per batch
_TWC = _T * _WC  # 988000

_NSB = 2  # batches on the SBUF/SWDGE path; the rest go direct HWDGE

_PPB = 100  # partitions per batch
_RPP = 20  # output rows per partition  (100 * 20 = 2000)
_STEP = _RPP * _C  # 520: flat-input stride between partition slices
_SEG = _STEP + (_WC - _C)  # 988: slice length incl. 468-element halo
_FI = _NSB * _SEG  # free elems/partition, input tile
_RW = _RPP * _WC  # 9880: dense output elems/partition/batch
_FO = _NSB * _RW  # free elems/partition, output tile

_nc_cache = None


def _build():
    global _nc_cache
    if _nc_cache is not None:
        return _nc_cache
    nc = bass.Bass()
    xp = nc.declare_dram_parameter("xp", [_BPC, _NP], _BF16, isOutput=False)
    out = nc.declare_dram_parameter("out", [_BPC, _T, _WC], _BF16, isOutput=True)

    with (
        nc.sbuf_tensor([128, _FI], _BF16) as tin,
        nc.sbuf_tensor([128, _FO], _BF16) as tout,
        nc.Block() as block,
        nc.semaphore("l0") as l0,
        nc.semaphore("l1") as l1,
        nc.semaphore("es") as es,
        nc.semaphore("ss") as ss,
        nc.semaphore("hs") as hs,
    ):
        lsem = [l0, l1]

        @block.gpsimd
        def _(e):
            for b in range(_NSB):
                e.dma_start(
                    out=AP(tin, b * _SEG, [[_FI, _PPB], [1, _SEG]]),
                    in_=AP(xp, b * _NP, [[_STEP, _PPB], [1, _SEG]]),
                ).then_inc(lsem[b], 16)
            for b in range(_NSB):
                e.wait_ge(es, b + 1)
                e.dma_start(
                    out=AP(out, b * _TWC, [[_RW, _PPB], [1, _RW]]),
                    in_=AP(tout, b * _RW, [[_FO, _PPB], [1, _RW]]),
                ).then_inc(ss, 16)
            e.wait_ge(ss, 16 * _NSB)

        @block.vector
        def _(v):
            for b in range(_NSB):
                v.wait_ge(lsem[b], 16)
                v.tensor_copy(
                    out=AP(
                        tout, b * _RW, [[_FO, _PPB], [_WC, _RPP], [1, _WC]]
                    ).bitcast(_I32),
                    in_=AP(
                        tin, b * _SEG, [[_FI, _PPB], [_C, _RPP], [1, _WC]]
                    ).bitcast(_I32),
                ).then_inc(es, 1)

        @block.sync
        def _(e):
            b = _NSB  # batch 2: direct DRAM->DRAM window gather
            e.dma_start(
                out=AP(out, b * _TWC, [[_WC, _T], [1, _WC]]),
                in_=AP(xp, b * _NP, [[_C, _T], [1, _WC]]),
            ).then_inc(hs, 16)
            e.wait_ge(hs, 32)

        @block.scalar
        def _(e):
            b = _NSB + 1  # batch 3: direct DRAM->DRAM window gather
            e.dma_start(
                out=AP(out, b * _TWC, [[_WC, _T], [1, _WC]]),
                in_=AP(xp, b * _NP, [[_C, _T], [1, _WC]]),
            ).then_inc(hs, 16)
            e.wait_ge(hs, 32)

    _nc_cache = nc
    return nc


def _make_in_maps(x: np.ndarray) -> list[dict]:
    """x: [B, T, C] float32 -> per-core padded bf16 flat inputs."""
    xb = np.asarray(x, dtype=np.float32).astype(_NPBF16)
    xpad = np.zeros((_B, _NP), _NPBF16)
    xpad[:, _PAD : _PAD + _T * _C] = xb.reshape(_B, _T * _C)
    return [
        {"xp": np.ascontiguousarray(xpad[i * _BPC : (i + 1) * _BPC])}
        for i in range(_NCORES)
    ]


def _gather_out(results) -> np.ndarray:
    return np.concatenate(
        [np.asarray(r["out"]).astype(np.float32) for r in results], axis=0
    ).reshape(_B, _T, _WC)


def kernel(x: np.ndarray) -> np.ndarray:
    assert np.asarray(x).shape == (_B, _T, _C)
    nc = _build()
    res = run_bass_kernel_spmd(nc, _make_in_maps(x), list(range(_NCORES)))
    return _gather_out(res.results)


# revision 15
# speedup vs baseline: 1.2625x; 1.1366x over previous
"""Trainium2 Bass kernel for nn_CreateOverlappingWindows.

out[b, t, w*C + c] = x_padded[b, t + w, c]  (SAME zero padding, n_context=9)

Flattening (w, c) -> 494 contiguous values, each output row is a contiguous
494-element window of the zero-padded flattened input:
    out[b, t, :] = xpad_flat[b, t*C : t*C + W*C]

Strategy (memory-regime): bf16 end-to-end (tolerance 2e-2; bf16 keeps f32's
exponent range so rel err <= 2^-9 ~ 2e-3), and run TWO independent
output-write paths concurrently, each bound by a different resource:

  * SBUF path (batches 0,1): SWDGE loads the padded input into SBUF
    (100 partitions x 988-elem overlapping slices), DVE expands the windows
    into a dense tile (int32-viewed copies, ~2.7us/batch), SWDGE streams it
    out with 19.8 KB fully-contiguous descriptors (the per-descriptor
    throughput sweet spot; >~20KB descriptors run at half rate).
  * Direct path (batches 2,3): one HWDGE ring each (sync / scalar) does
    the window-gather DRAM->DRAM with 988 B descriptors straight from the
    padded input (HWDGE descriptor-rate bound, ~10.4ns/desc -> ~21us).

Sharding: pure data parallel - batch 32 split 4-per-core across 8 cores.
"""

import sys

sys.path.insert(0, "/opt/trn_rl_repo")

import ml_dtypes
import numpy as np
from concourse import bass, mybir
from concourse.ap import AP
from concourse.bass_utils import run_bass_kernel_spmd

_BF16 = mybir.dt.bfloat16
_I32 = mybir.dt.int32
_NPBF16 = ml_dtypes.bfloat16

_NCORES = 8
_B, _T, _C = 32, 2000, 26
_NCTX = 9
_W = 2 * _NCTX + 1  # 19
_WC = _W * _C  # 494
_PAD = _NCTX * _C  # 234
_BPC = _B // _NCORES  # 4 batches per core
_NP = _T * _C + 2 * _PAD  # 52468 padded flat length per batch
_TWC = _T * _WC  # 988000

_NSB = 2  # batches on the SBUF/SWDGE path; the rest go direct HWDGE

_PPB = 100  # partitions per batch
_RPP = 20  # output rows per partition  (100 * 20 = 2000)
_STEP = _RPP * _C  # 520: flat-input stride between partition slices
_SEG = _STEP + (_WC - _C)  # 988: slice length incl. 468-element halo
_FI = _NSB * _SEG  # free elems/partition, input tile
_RW = _RPP * _WC  # 9880: dense output elems/partition/batch
_FO = _NSB * _RW  # free elems/partition, output tile

_nc_cache = None


def _build():
    global _nc_cache
    if _nc_cache is not None:
        return _nc_cache
    nc = bass.Bass()
    xp = nc.declare_dram_parameter("xp", [_BPC, _NP], _BF16, isOutput=False)
    out = nc.declare_dram_parameter("out", [_BPC, _T, _WC], _BF16, isOutput=True)

    with (
        nc.sbuf_tensor([128, _FI], _BF16) as tin,
        nc.sbuf_tensor([128, _FO], _BF16) as tout,
        nc.Block() as block,
        nc.semaphore("l0") as l0,
        nc.semaphore("l1") as l1,
        nc.semaphore("es") as es,
        nc.semaphore("ss") as ss,
        nc.semaphore("hs") as hs,
    ):
        lsem = [l0, l1]

        @block.gpsimd
        def _(e):
            for b in range(_NSB):
                e.dma_start(
                    out=AP(tin, b * _SEG, [[_FI, _PPB], [1, _SEG]]),
                    in_=AP(xp, b * _NP, [[_STEP, _PPB], [1, _SEG]]),
                ).then_inc(lsem[b], 16)
            for b in range(_NSB):
                e.wait_ge(es, b + 1)
                e.dma_start(
                    out=AP(out, b * _TWC, [[_RW, _PPB], [1, _RW]]),
                    in_=AP(tout, b * _RW, [[_FO, _PPB], [1, _RW]]),
                ).then_inc(ss, 16)
            e.wait_ge(ss, 16 * _NSB)

        @block.vector
        def _(v):
            for b in range(_NSB):
                v.wait_ge(lsem[b], 16)
                v.tensor_copy(
                    out=AP(
                        tout, b * _RW, [[_FO, _PPB], [_WC, _RPP], [1, _WC]]
                    ).bitcast(_I32),
                    in_=AP(
                        tin, b * _SEG, [[_FI, _PPB], [_C, _RPP], [1, _WC]]
                    ).bitcast(_I32),
                ).then_inc(es, 1)

        @block.sync
        def _(e):
            # let the (tiny) SWDGE loads through the engines first
            e.wait_ge(l0, 16)
            e.wait_ge(l1, 16)
            b = _NSB  # batch 2: direct DRAM->DRAM window gather
            e.dma_start(
                out=AP(out, b * _TWC, [[_WC, _T], [1, _WC]]),
                in_=AP(xp, b * _NP, [[_C, _T], [1, _WC]]),
            ).then_inc(hs, 16)
            e.wait_ge(hs, 32)

        @block.scalar
        def _(e):
            e.wait_ge(l0, 16)
            e.wait_ge(l1, 16)
            b = _NSB + 1  # batch 3: direct DRAM->DRAM window gather
            e.dma_start(
                out=AP(out, b * _TWC, [[_WC, _T], [1, _WC]]),
                in_=AP(xp, b * _NP, [[_C, _T], [1, _WC]]),
            ).then_inc(hs, 16)
            e.wait_ge(hs, 32)

    _nc_cache = nc
    return nc


def _make_in_maps(x: np.ndarray) -> list[dict]:
    """x: [B, T, C] float32 -> per-core padded bf16 flat inputs."""
    xb = np.asarray(x, dtype=np.float32).astype(_NPBF16)
    xpad = np.zeros((_B, _NP), _NPBF16)
    xpad[:, _PAD : _PAD + _T * _C] = xb.reshape(_B, _T * _C)
    return [
        {"xp": np.ascontiguousarray(xpad[i * _BPC : (i + 1) * _BPC])}
        for i in range(_NCORES)
    ]


def _gather_out(results) -> np.ndarray:
    return np.concatenate(
        [np.asarray(r["out"]).astype(np.float32) for r in results], axis=0
    ).reshape(_B, _T, _WC)


def kernel(x: np.ndarray) -> np.ndarray:
    assert np.asarray(x).shape == (_B, _T, _C)
    nc = _build()
    res = run_bass_kernel_spmd(nc, _make_in_maps(x), list(range(_NCORES)))
    return _gather_out(res.results)
